# revision 13
# baseline (speedup 1.0000x reference)
"""ALiBi positional bias: out[b,h,i,j] = scores[b,h,i,j] - slope[h] * |j - i|.

Full input [2, 16, 2048, 2048] f32. Sharded over heads across 8 NeuronCores:
core c handles heads {2c, 2c+1} for both batches (shard [2, 2, 2048, 2048]).
Pure elementwise add — no communication. The bias is generated on-device
(iota + min trick) so HBM traffic is just scores in + out.

Per 128-row block:
  iota (POOL):  v[p, j] = j - (i0 + p)
  DVE stt:      vneg = (v * -1) min v = -|j - i|
  per (b,h):    out = (vneg * slope_h) + x   — one fused DVE op per tile
"""

import math
import os

import numpy as np

import concourse.bacc as bacc
import concourse.mybir as mybir
from concourse.bass import AP
from concourse.bass_utils import run_bass_kernel_spmd
from concourse.tile import TileContext

B = 2
H = 16
S = 2048
N_CORES = 8
HEADS_PER_CORE = H // N_CORES  # 2
PAIRS = B * HEADS_PER_CORE  # 4 (b, h_local) pairs per core
P = 128  # partitions
BLOCKS = S // P  # 16 row blocks per matrix

LAST_RESULTS = None  # BassKernelResults from the most recent run (for test harness)


def _alibi_slopes(num_heads: int) -> np.ndarray:
    def pow2_slopes(n):
        start = 2.0 ** (-(2.0 ** (-(math.log2(n) - 3))))
        return [start * (start**i) for i in range(n)]

    if math.log2(num_heads).is_integer():
        slopes = pow2_slopes(num_heads)
    else:
        cp2 = 2 ** int(math.floor(math.log2(num_heads)))
        slopes = pow2_slopes(cp2)
        extra = [
            slopes[0] * 2.0 ** (-(i - 1) / (num_heads - cp2))
            for i in range(1, num_heads - cp2 + 1)
        ]
        slopes = slopes + extra
    return np.array(slopes, dtype=np.float32)


def _build():
    nc = bacc.Bacc(None, target_bir_lowering=False)
    x = nc.declare_dram_parameter("x", [B, HEADS_PER_CORE, S, S], mybir.dt.float32, isOutput=False)
    slopes = nc.declare_dram_parameter("slopes", [P, HEADS_PER_CORE], mybir.dt.float32, isOutput=False)
    out = nc.declare_dram_parameter("out", [B, HEADS_PER_CORE, S, S], mybir.dt.float32, isOutput=True)

    with TileContext(nc) as tc:
        with (
            tc.tile_pool(name="const", bufs=1) as cpool,
            tc.tile_pool(name="xt", bufs=14) as xpool,
        ):
            slopes_sb = cpool.tile([P, HEADS_PER_CORE], mybir.dt.float32)
            nc.gpsimd.dma_start(out=slopes_sb[:, :], in_=slopes[:, :])

            # Master relative-position tile, computed once (in place):
            #   M[p, k] = -|k - S - p|,  k in [0, 2S)
            # Every block's bias input is the slice M[:, S - i0 : 2S - i0]:
            #   M[p, (j - i0) + S] = -|j - i0 - p|
            # Built high half first: block t=0 only needs M[:, S:2S], so the
            # first bias add can start after half the iota/min latency.
            m = cpool.tile([P, 2 * S], mybir.dt.float32, tag="m")
            for lo, base in ((S, 0), (0, -S)):
                half = m[:, lo : lo + S]
                nc.gpsimd.iota(
                    half,
                    pattern=[[1, S]],
                    base=base,
                    channel_multiplier=-1,
                    allow_small_or_imprecise_dtypes=True,
                )
                nc.vector.scalar_tensor_tensor(
                    out=half,
                    in0=half,
                    scalar=-1.0,
                    in1=half,
                    op0=mybir.AluOpType.mult,
                    op1=mybir.AluOpType.min,
                )

            for t in range(BLOCKS):
                i0 = t * P
                rel = m[:, S - i0 : 2 * S - i0]
                for p in range(PAIRS):
                    b, h = divmod(p, HEADS_PER_CORE)
                    xt = xpool.tile([P, S], mybir.dt.float32)
                    nc.scalar.dma_start(out=xt[:, :], in_=x[b, h, i0 : i0 + P, :])
                    # xt = (rel * slope_h) + xt   — one fused DVE op
                    nc.vector.scalar_tensor_tensor(
                        out=xt[:, :],
                        in0=rel,
                        scalar=slopes_sb[:, h : h + 1],
                        in1=xt[:, :],
                        op0=mybir.AluOpType.mult,
                        op1=mybir.AluOpType.add,
                    )
                    nc.sync.dma_start(out=out[b, h, i0 : i0 + P, :], in_=xt[:, :])
    nc.compile()
    return nc


def kernel(attention_scores: np.ndarray) -> np.ndarray:
    global LAST_RESULTS
    scores = np.asarray(attention_scores, dtype=np.float32)
    assert scores.shape == (B, H, S, S), scores.shape

    slopes_all = _alibi_slopes(H)  # [H], positive

    nc = _build()

    in_maps = []
    for c in range(N_CORES):
        h0 = c * HEADS_PER_CORE
        shard = scores[:, h0 : h0 + HEADS_PER_CORE]  # [B, HPC, S, S] view
        head_slopes = slopes_all[h0 : h0 + HEADS_PER_CORE].astype(np.float32)
        slopes_bcast = np.tile(head_slopes[None, :], (P, 1))  # [128, HPC]
        in_maps.append({"x": shard, "slopes": slopes_bcast})

    trace = bool(int(os.environ.get("ALIBI_TRACE", "0")))
    LAST_RESULTS = run_bass_kernel_spmd(
        nc, in_maps, core_ids=list(range(N_CORES)), trace=trace
    )

    full = np.empty((B, H, S, S), dtype=np.float32)
    for c in range(N_CORES):
        h0 = c * HEADS_PER_CORE
        full[:, h0 : h0 + HEADS_PER_CORE] = LAST_RESULTS.results[c]["out"]
    return full


# revision 14
# speedup vs baseline: 1.0798x; 1.0798x over previous
"""ALiBi positional bias: out[b,h,i,j] = scores[b,h,i,j] - slope[h] * |j - i|.

Full input [2, 16, 2048, 2048] f32. Sharded over heads across 8 NeuronCores:
core c handles heads {2c, 2c+1} for both batches (shard [2, 2, 2048, 2048]).
Pure elementwise add — no communication. The bias is generated on-device
(iota + min trick) so HBM traffic is just scores in + out.

Per 128-row block:
  iota (POOL):  v[p, j] = j - (i0 + p)
  DVE stt:      vneg = (v * -1) min v = -|j - i|
  per (b,h):    out = (vneg * slope_h) + x   — one fused DVE op per tile
"""

import math
import os

import numpy as np

import concourse.bacc as bacc
import concourse.mybir as mybir
from concourse.bass import AP
from concourse.bass_utils import run_bass_kernel_spmd
from concourse.tile import TileContext

B = 2
H = 16
S = 2048
N_CORES = 8
HEADS_PER_CORE = H // N_CORES  # 2
PAIRS = B * HEADS_PER_CORE  # 4 (b, h_local) pairs per core
P = 128  # partitions
BLOCKS = S // P  # 16 row blocks per matrix

LAST_RESULTS = None  # BassKernelResults from the most recent run (for test harness)


def _alibi_slopes(num_heads: int) -> np.ndarray:
    def pow2_slopes(n):
        start = 2.0 ** (-(2.0 ** (-(math.log2(n) - 3))))
        return [start * (start**i) for i in range(n)]

    if math.log2(num_heads).is_integer():
        slopes = pow2_slopes(num_heads)
    else:
        cp2 = 2 ** int(math.floor(math.log2(num_heads)))
        slopes = pow2_slopes(cp2)
        extra = [
            slopes[0] * 2.0 ** (-(i - 1) / (num_heads - cp2))
            for i in range(1, num_heads - cp2 + 1)
        ]
        slopes = slopes + extra
    return np.array(slopes, dtype=np.float32)


def _build():
    nc = bacc.Bacc(None, target_bir_lowering=False)
    x = nc.declare_dram_parameter("x", [B, HEADS_PER_CORE, S, S], mybir.dt.float32, isOutput=False)
    slopes = nc.declare_dram_parameter("slopes", [P, HEADS_PER_CORE], mybir.dt.float32, isOutput=False)
    out = nc.declare_dram_parameter("out", [B, HEADS_PER_CORE, S, S], mybir.dt.float32, isOutput=True)

    with TileContext(nc) as tc:
        with (
            tc.tile_pool(name="const", bufs=1) as cpool,
            tc.tile_pool(name="xt", bufs=12) as xpool,
        ):
            slopes_sb = cpool.tile([P, HEADS_PER_CORE], mybir.dt.float32)
            nc.gpsimd.dma_start(out=slopes_sb[:, :], in_=slopes[:, :])

            # Master relative-position tile, computed once (in place):
            #   M[p, k] = -|k - S - p|,  k in [0, 2S)
            # Every block's bias input is the slice M[:, S - i0 : 2S - i0]:
            #   M[p, (j - i0) + S] = -|j - i0 - p|
            # Built high half first: block t=0 only needs M[:, S:2S], so the
            # first bias add can start after half the iota/min latency.
            m = cpool.tile([P, 2 * S], mybir.dt.float32, tag="m")
            for lo, base in ((S, 0), (0, -S)):
                half = m[:, lo : lo + S]
                nc.gpsimd.iota(
                    half,
                    pattern=[[1, S]],
                    base=base,
                    channel_multiplier=-1,
                    allow_small_or_imprecise_dtypes=True,
                )
                nc.vector.scalar_tensor_tensor(
                    out=half,
                    in0=half,
                    scalar=-1.0,
                    in1=half,
                    op0=mybir.AluOpType.mult,
                    op1=mybir.AluOpType.min,
                )

            for t in range(BLOCKS):
                i0 = t * P
                rel = m[:, S - i0 : 2 * S - i0]
                for p in range(PAIRS):
                    b, h = divmod(p, HEADS_PER_CORE)
                    xt = xpool.tile([P, S], mybir.dt.float32)
                    nc.scalar.dma_start(out=xt[:, :], in_=x[b, h, i0 : i0 + P, :])
                    # xt = (rel * slope_h) + xt   — one fused DVE op
                    nc.vector.scalar_tensor_tensor(
                        out=xt[:, :],
                        in0=rel,
                        scalar=slopes_sb[:, h : h + 1],
                        in1=xt[:, :],
                        op0=mybir.AluOpType.mult,
                        op1=mybir.AluOpType.add,
                    )
                    nc.sync.dma_start(out=out[b, h, i0 : i0 + P, :], in_=xt[:, :])
    nc.compile()
    return nc


def kernel(attention_scores: np.ndarray) -> np.ndarray:
    global LAST_RESULTS
    scores = np.asarray(attention_scores, dtype=np.float32)
    assert scores.shape == (B, H, S, S), scores.shape

    slopes_all = _alibi_slopes(H)  # [H], positive

    nc = _build()

    in_maps = []
    for c in range(N_CORES):
        h0 = c * HEADS_PER_CORE
        shard = scores[:, h0 : h0 + HEADS_PER_CORE]  # [B, HPC, S, S] view
        head_slopes = slopes_all[h0 : h0 + HEADS_PER_CORE].astype(np.float32)
        slopes_bcast = np.tile(head_slopes[None, :], (P, 1))  # [128, HPC]
        in_maps.append({"x": shard, "slopes": slopes_bcast})

    trace = bool(int(os.environ.get("ALIBI_TRACE", "0")))
    LAST_RESULTS = run_bass_kernel_spmd(
        nc, in_maps, core_ids=list(range(N_CORES)), trace=trace
    )

    full = np.empty((B, H, S, S), dtype=np.float32)
    for c in range(N_CORES):
        h0 = c * HEADS_PER_CORE
        full[:, h0 : h0 + HEADS_PER_CORE] = LAST_RESULTS.results[c]["out"]
    return full


# revision 15
# speedup vs baseline: 1.2177x; 1.1277x over previous
"""ALiBi positional bias: out[b,h,i,j] = scores[b,h,i,j] - slope[h] * |j - i|.

Full input [2, 16, 2048, 2048] f32. Sharded over heads across 8 NeuronCores:
core c handles heads {2c, 2c+1} for both batches (shard [2, 2, 2048, 2048],
64 MiB in + 64 MiB out per core). Pure elementwise add — no communication.

The bias is generated on-device so HBM traffic is exactly scores-in + out:
  - One master tile M[p, k] = -|k - S - 2p| (k in [0, 2S)) built once via
    GpSimd iota + a DVE (v*-1) min v trick (f32 iota is exact: |v| < 2^24).
  - Tiles hold 2 consecutive rows per partition (16 KiB contiguous per
    partition, 2 MiB contiguous per DMA): partition p covers rows
    r = i0 + 2p + s, s in {0,1}.  Then -|j - r| = M[p, (S - i0) - s + j],
    an overlapping-window AP with free steps [s: -1][j: +1].
  - One fused DVE scalar_tensor_tensor per tile: (M_slice * slope_h) + x.

Measured (solo core): ~327 us; HBM stream runs gapless at ~423 GB/s
(97% of the 436 GB/s per-core SBUF AXI fabric limit). Bit-exact vs the
jax reference (rel err 0.0).
"""

import math
import os

import numpy as np

import concourse.bacc as bacc
import concourse.mybir as mybir
from concourse.bass import AP
from concourse.bass_utils import run_bass_kernel_spmd
from concourse.tile import TileContext

B = 2
H = 16
S = 2048
N_CORES = 8
HEADS_PER_CORE = H // N_CORES  # 2
PAIRS = B * HEADS_PER_CORE  # 4 (b, h_local) pairs per core
P = 128  # partitions
ROWS = 2 * P  # rows per tile: partition p holds rows i0 + 2p + {0, 1}

LAST_RESULTS = None  # BassKernelResults from the most recent run (for test harness)


def _alibi_slopes(num_heads: int) -> np.ndarray:
    def pow2_slopes(n):
        start = 2.0 ** (-(2.0 ** (-(math.log2(n) - 3))))
        return [start * (start**i) for i in range(n)]

    if math.log2(num_heads).is_integer():
        slopes = pow2_slopes(num_heads)
    else:
        cp2 = 2 ** int(math.floor(math.log2(num_heads)))
        slopes = pow2_slopes(cp2)
        extra = [
            slopes[0] * 2.0 ** (-(i - 1) / (num_heads - cp2))
            for i in range(1, num_heads - cp2 + 1)
        ]
        slopes = slopes + extra
    return np.array(slopes, dtype=np.float32)


def _build():
    nc = bacc.Bacc(None, target_bir_lowering=False)
    x = nc.declare_dram_parameter("x", [B, HEADS_PER_CORE, S, S], mybir.dt.float32, isOutput=False)
    slopes = nc.declare_dram_parameter("slopes", [P, HEADS_PER_CORE], mybir.dt.float32, isOutput=False)
    out = nc.declare_dram_parameter("out", [B, HEADS_PER_CORE, S, S], mybir.dt.float32, isOutput=True)

    with TileContext(nc) as tc:
        with (
            tc.tile_pool(name="const", bufs=1) as cpool,
            tc.tile_pool(name="xt", bufs=10) as xpool,
        ):
            slopes_sb = cpool.tile([P, HEADS_PER_CORE], mybir.dt.float32)
            nc.gpsimd.dma_start(out=slopes_sb[:, :], in_=slopes[:, :])

            # Master relative-position tile, computed once (in place):
            #   M[p, k] = -|k - S - 2p|,  k in [0, 2S)
            # Built high half first: block t=0 only needs M[:, S-1:2S], so
            # the first bias add is ready after half the iota/min latency.
            m = cpool.tile([P, 2 * S], mybir.dt.float32, tag="m")
            for lo, base in ((S, 0), (0, -S)):
                half = m[:, lo : lo + S]
                nc.gpsimd.iota(
                    half,
                    pattern=[[1, S]],
                    base=base,
                    channel_multiplier=-2,
                    allow_small_or_imprecise_dtypes=True,
                )
                nc.vector.scalar_tensor_tensor(
                    out=half,
                    in0=half,
                    scalar=-1.0,
                    in1=half,
                    op0=mybir.AluOpType.mult,
                    op1=mybir.AluOpType.min,
                )
            m_handle = m[:, 0:1].tensor

            for t in range(S // ROWS):
                i0 = t * ROWS
                # rel[p, s, j] = M[p, (S - i0) - s + j] = -|j - (i0 + 2p + s)|
                rel = AP(m_handle, S - i0, [[2 * S, P], [-1, 2], [1, S]])
                for p in range(PAIRS):
                    b, h = divmod(p, HEADS_PER_CORE)
                    xt = xpool.tile([P, 2, S], mybir.dt.float32)
                    nc.scalar.dma_start(
                        out=xt[:, :, :],
                        in_=x[b, h, i0 : i0 + ROWS, :].rearrange(
                            "(p s) j -> p s j", s=2
                        ),
                    )
                    # xt = (rel * slope_h) + xt   — one fused DVE op
                    nc.vector.scalar_tensor_tensor(
                        out=xt[:, :, :],
                        in0=rel,
                        scalar=slopes_sb[:, h : h + 1],
                        in1=xt[:, :, :],
                        op0=mybir.AluOpType.mult,
                        op1=mybir.AluOpType.add,
                    )
                    nc.sync.dma_start(
                        out=out[b, h, i0 : i0 + ROWS, :].rearrange(
                            "(p s) j -> p s j", s=2
                        ),
                        in_=xt[:, :, :],
                    )
    nc.compile()
    return nc


def kernel(attention_scores: np.ndarray) -> np.ndarray:
    global LAST_RESULTS
    scores = np.asarray(attention_scores, dtype=np.float32)
    assert scores.shape == (B, H, S, S), scores.shape

    slopes_all = _alibi_slopes(H)  # [H], positive

    nc = _build()

    in_maps = []
    for c in range(N_CORES):
        h0 = c * HEADS_PER_CORE
        shard = scores[:, h0 : h0 + HEADS_PER_CORE]  # [B, HPC, S, S] view
        head_slopes = slopes_all[h0 : h0 + HEADS_PER_CORE].astype(np.float32)
        slopes_bcast = np.tile(head_slopes[None, :], (P, 1))  # [128, HPC]
        in_maps.append({"x": shard, "slopes": slopes_bcast})

    trace = bool(int(os.environ.get("ALIBI_TRACE", "0")))
    LAST_RESULTS = run_bass_kernel_spmd(
        nc, in_maps, core_ids=list(range(N_CORES)), trace=trace
    )

    full = np.empty((B, H, S, S), dtype=np.float32)
    for c in range(N_CORES):
        h0 = c * HEADS_PER_CORE
        full[:, h0 : h0 + HEADS_PER_CORE] = LAST_RESULTS.results[c]["out"]
    return full
